# revision 15
# baseline (speedup 1.0000x reference)
"""ChebNet (K=3, L=2) forward on 8 Trainium2 NeuronCores — v2.

Node-sharded SPMD: each core owns 6250 dst rows, placed into 6784 padded
columns (53 blocks x 128) by a 2-D greedy packing that balances each
32-col window's in-edge count per source half (halves = core groups 0-3 /
4-7, each half's table slice int16-addressable). Chebyshev recurrence in
the U-basis (U1 = A_hat U0, U2 = A_hat U1; out = U0(W0-W2) - U1 W1 +
2 U2 W2).

SpMM: per PSUM chunk (16 windows = 512 cols), one dma_gather per half
(up to 4096 edge slots; 2 tiles of 128 slots per window-half, static) +
PE mask-matmuls with DVE-generated masks carrying norm[src]*norm[dst].
Table layout is p-major (table row = p*53 + k for column k*128+p) so the
post-SpMM transpose writes DRAM with 128 fat descriptors. idx/dcol/wval
stay SBUF-resident. Transposes are fused into the producing SpMM's chunk
loop; 3 AllGathers rebuild the node table between SpMMs.
"""
import os
import numpy as np

N, E, D, K, L = 50000, 800000, 64, 3, 2
NCORES = 8
NSH = N // NCORES              # 6250 owned nodes per core
NBLK = 53                      # 128-col blocks per core
NCOLS = NBLK * 128             # 6784 padded columns
WIN = 32                       # dst cols per window
NWIN = NCOLS // WIN            # 212
TPW = 2                        # tiles per (window, half) — static (cap 256)
TBL_ROWS = NCORES * NCOLS      # 54272
HALF = TBL_ROWS // 2           # 27136 rows per half (int16-reachable)
CWIN = 16                      # windows per PSUM chunk
CHUNK_WINS = [CWIN] * 13 + [NWIN - 13 * CWIN]   # 13x16 + 1x4
NCH = len(CHUNK_WINS)          # 14
SLOTS_W = TPW * 128            # 256 slots per window-half
CH_SLOT_BASE = np.concatenate([[0], np.cumsum([2 * nw * SLOTS_W for nw in CHUNK_WINS])]).astype(int)
CH_TILE_BASE = np.concatenate([[0], np.cumsum([2 * nw * TPW for nw in CHUNK_WINS])]).astype(int)
CH_COL_BASE = np.concatenate([[0], np.cumsum([nw * WIN for nw in CHUNK_WINS])]).astype(int)
SLOTS_TOT = int(CH_SLOT_BASE[-1])   # 108544
NTILE_TOT = SLOTS_TOT // 128        # 848
GMAX = int(os.environ.get("KGMAX", "1024"))  # max rows per dma_gather call
                                             # (>1024 overflows the SWDGE ring)
NQ = int(os.environ.get("KQ", "4"))          # SWDGE queues to rotate over

_CACHE = {}


# ---------------------------------------------------------------------------
# Workaround for this walrus build: any instruction carrying >1 sync wait is
# rejected ("Too many sync wait commands"). Hoist extras onto 1-wait NoOps on
# the same engine (per-engine program order preserves semantics).
_ws_counter = [0]


def _split_multiwaits(nc):
    import concourse.mybir as mybir
    n_split = 0
    for fn in nc.m.functions:
        for bb in fn.blocks:
            new_list = []
            changed = False
            for inst in bb.instructions:
                si = inst.sync_info
                waits = list(si.on_wait) if si is not None else []
                if len(waits) > 1:
                    changed = True
                    for w in waits[:-1]:
                        _ws_counter[0] += 1
                        nop = mybir.InstNoOp(
                            name=f"waitsplit-{_ws_counter[0]}",
                            ins=[], outs=[],
                            sync_info=mybir.SyncInfo(on_wait=[w], on_update=[]),
                        )
                        nop.engine = inst.engine
                        nc.register_instruction(nop, overwrite=True)
                        new_list.append(nop)
                        n_split += 1
                    si.on_wait = waits[-1:]
                new_list.append(inst)
            if changed:
                bb.instructions[:] = new_list
    return n_split


def _finalize_with_split(nc):
    import concourse.bass as _bass
    nc.compile()           # Bacc passes (incl. library-load insertion)
    _split_multiwaits(nc)  # after replace_nops_with_events, before freeze
    _bass.Bass.finalize(nc)


def _build_runner(nc, n_cores):
    """SPMD runner over the axon PJRT backend (keeps the jitted executable
    and device-resident inputs so repeat calls can be timed)."""
    import jax
    from jax.sharding import Mesh, PartitionSpec
    from jax.experimental.shard_map import shard_map
    import concourse.mybir as mybir
    from concourse.bass2jax import (
        _bass_exec_p, install_neuronx_cc_hook, partition_id_tensor)

    install_neuronx_cc_hook()
    partition_name = nc.partition_id_tensor.name if nc.partition_id_tensor else None

    in_names, out_names, out_avals, zero_outs = [], [], [], []
    for alloc in nc.m.functions[0].allocations:
        if not isinstance(alloc, mybir.MemoryLocationSet):
            continue
        name = alloc.memorylocations[0].name
        if alloc.kind == "ExternalInput":
            if name != partition_name:
                in_names.append(name)
        elif alloc.kind == "ExternalOutput":
            shape = tuple(alloc.tensor_shape)
            dtype = mybir.dt.np(alloc.dtype)
            out_names.append(name)
            out_avals.append(jax.core.ShapedArray(shape, dtype))
            zero_outs.append(np.zeros(shape, dtype))
    n_params = len(in_names)
    all_in_names = list(in_names) + list(out_names)
    if partition_name is not None:
        all_in_names.append(partition_name)

    def _body(*args):
        operands = list(args)
        if partition_name is not None:
            operands.append(partition_id_tensor())
        outs = _bass_exec_p.bind(
            *operands,
            out_avals=tuple(out_avals),
            in_names=tuple(all_in_names),
            out_names=tuple(out_names),
            lowering_input_output_aliases=(),
            sim_require_finite=True,
            sim_require_nnan=True,
            nc=nc,
        )
        return tuple(outs)

    devices = jax.devices()[:n_cores]
    mesh = Mesh(np.asarray(devices), ("core",))
    in_specs = (PartitionSpec("core"),) * (n_params + len(out_names))
    out_specs = (PartitionSpec("core"),) * len(out_names)
    sharded = jax.jit(
        shard_map(_body, mesh=mesh, in_specs=in_specs, out_specs=out_specs,
                  check_rep=False),
        keep_unused=True,
    )

    def run(in_maps, iters=1):
        import time as _time
        per_core = [[np.asarray(m[name]) for name in in_names] for m in in_maps]
        concat_in = [
            np.concatenate([per_core[c][i] for c in range(n_cores)], axis=0)
            for i in range(n_params)
        ]
        concat_zeros = [
            np.zeros((n_cores * z.shape[0], *z.shape[1:]), z.dtype)
            for z in zero_outs
        ]
        sharding = jax.sharding.NamedSharding(mesh, PartitionSpec("core"))
        dev_in = [jax.device_put(a, sharding) for a in concat_in + concat_zeros]
        out = sharded(*dev_in)
        jax.block_until_ready(out)
        times = []
        for _ in range(iters):
            t0 = _time.perf_counter()
            out = sharded(*dev_in)
            jax.block_until_ready(out)
            times.append(_time.perf_counter() - t0)
        results = [
            {name: np.asarray(out[i]).reshape(n_cores, *out_avals[i].shape)[c]
             for i, name in enumerate(out_names)}
            for c in range(n_cores)
        ]
        return results, times

    return run


def _pack_windows(degA, degB):
    """Greedy 2-D balanced packing of NSH nodes into NWIN windows of <=32,
    minimizing the max per-half load. Returns win_of[node]."""
    order = np.argsort(-(degA + degB), kind="stable")
    loadA = np.zeros(NWIN)
    loadB = np.zeros(NWIN)
    cnt = np.zeros(NWIN, np.int64)
    win_of = np.zeros(degA.size, np.int64)
    for v in order:
        av, bv = degA[v], degB[v]
        cand = np.maximum(loadA + av, loadB + bv) + 1e-3 * (loadA + loadB)
        cand[cnt >= 32] = np.inf
        w = int(np.argmin(cand))
        win_of[v] = w
        loadA[w] += av
        loadB[w] += bv
        cnt[w] += 1
    if loadA.max() > SLOTS_W or loadB.max() > SLOTS_W:
        raise RuntimeError(f"window overflow: {loadA.max()}, {loadB.max()}")
    return win_of


def _host_prep(features, src, dst, W, b, pw, pb):
    src = np.asarray(src).astype(np.int64)
    dst = np.asarray(dst).astype(np.int64)
    features = np.asarray(features, dtype=np.float32)
    W = np.asarray(W, dtype=np.float32)
    b = np.asarray(b, dtype=np.float32)
    pw = np.asarray(pw, dtype=np.float32).reshape(D, 1)
    pb = np.asarray(pb, dtype=np.float32).reshape(1)

    deg = np.bincount(dst, minlength=N).astype(np.float32)
    norm = np.clip(deg, 1.0, None) ** -0.5
    wedge = (norm[src] * norm[dst]).astype(np.float32)

    half_of_src = (src >= N // 2).astype(np.int64)   # half = src core group
    degA = np.bincount(dst[half_of_src == 0], minlength=N)
    degB = np.bincount(dst[half_of_src == 1], minlength=N)

    # --- per-core window packing -> global column/table-row maps ----------
    col_of = np.zeros(N, np.int64)       # local column on owner core
    for i in range(NCORES):
        sl = slice(i * NSH, (i + 1) * NSH)
        win_of = _pack_windows(degA[sl].astype(np.float64),
                               degB[sl].astype(np.float64))
        # positions within each window in node order
        order = np.argsort(win_of, kind="stable")
        pos = np.arange(NSH) - np.searchsorted(win_of[order], win_of[order])
        lc = np.empty(NSH, np.int64)
        lc[order] = win_of[order] * WIN + pos
        col_of[sl] = lc
    core_of = np.arange(N) // NSH
    p_of = col_of % 128
    k_of = col_of // 128
    table_row = core_of * NCOLS + p_of * NBLK + k_of

    feat_pad = np.zeros((TBL_ROWS, D), dtype=np.float32)
    feat_pad[table_row] = features

    Wflat = np.zeros((D, L * 3 * D), dtype=np.float32)
    for l in range(L):
        for t, Wt in enumerate((W[l, 0] - W[l, 2], -W[l, 1], 2.0 * W[l, 2])):
            Wflat[:, (l * 3 + t) * D:(l * 3 + t + 1) * D] = Wt

    # window -> chunk map
    chunk_of_w = np.zeros(NWIN, np.int64)
    wl_of_w = np.zeros(NWIN, np.int64)
    w0 = 0
    for c, nw in enumerate(CHUNK_WINS):
        chunk_of_w[w0:w0 + nw] = c
        wl_of_w[w0:w0 + nw] = np.arange(nw)
        w0 += nw

    gather_sizes = []
    for c, nw in enumerate(CHUNK_WINS):
        G = nw * SLOTS_W
        for h in (0, 1):
            o = 0
            while o < G:
                gather_sizes.append(min(GMAX, G - o))
                o += GMAX

    in_maps = []
    for i in range(NCORES):
        sel = core_of[dst] == i
        e_src = src[sel]
        e_w = wedge[sel]
        lc_d = col_of[dst[sel]]
        win = lc_d // WIN
        dcol = (lc_d % WIN).astype(np.float32)
        h = half_of_src[sel]

        key = win * 2 + h
        order = np.argsort(key, kind="stable")
        ks = key[order]
        grp_start = np.searchsorted(ks, np.arange(NWIN * 2), side="left")
        grp_cnt = np.diff(np.append(grp_start, ks.size))
        if grp_cnt.max() > SLOTS_W:
            raise RuntimeError(f"slot overflow core {i}: {grp_cnt.max()}")
        rank = np.arange(ks.size) - grp_start[ks]

        ww = win[order]
        hh = h[order]
        c_ = chunk_of_w[ww]
        wl = wl_of_w[ww]
        nw_c = np.array(CHUNK_WINS)[c_]
        slot = (CH_SLOT_BASE[c_] + hh * nw_c * SLOTS_W + wl * SLOTS_W + rank)
        tile = (CH_TILE_BASE[c_] + hh * nw_c * TPW + wl * TPW + rank // 128)
        part = rank % 128

        idx_slots = np.zeros(SLOTS_TOT, dtype=np.int16)
        idx_slots[slot] = (table_row[e_src[order]] - hh * HALF).astype(np.int16)
        dcol_slots = np.full(SLOTS_TOT, -1.0, dtype=np.float32)
        wval_slots = np.zeros(SLOTS_TOT, dtype=np.float32)
        dcol_slots[tile * 128 + part] = dcol[order]
        wval_slots[tile * 128 + part] = e_w[order]

        # idx wrap: per gather, slot j -> idx_arr[j%16, base + j//16]
        idx_arr = np.zeros((16, SLOTS_TOT // 16), dtype=np.int16)
        pos = 0
        for G in gather_sizes:
            blk = idx_slots[pos:pos + G].reshape(G // 16, 16).T
            idx_arr[:, pos // 16:(pos + G) // 16] = blk
            pos += G
        idx_arr = np.tile(idx_arr, (8, 1))

        dcl = dcol_slots.reshape(NTILE_TOT, 128).T.copy()
        wvl = wval_slots.reshape(NTILE_TOT, 128).T.copy()

        f0T = np.zeros((D, NCOLS), dtype=np.float32)
        own = slice(i * NSH, (i + 1) * NSH)
        f0T[:, col_of[own]] = features[own].T

        iota = np.tile(np.arange(WIN, dtype=np.float32)[None, :], (128, 1))

        in_maps.append({
            "feat_pad": feat_pad,
            "f0T": f0T,
            "idx_all": idx_arr,
            "dcol": dcl,
            "wval": wvl,
            "iota": iota,
            "Wflat": Wflat,
            "bvec": b.T.copy(),
            "pwv": pw,
            "pbv": pb.reshape(1, 1),
        })
    _CACHE["col_of"] = col_of
    return in_maps


def _build_nc(repeat=1, mode="full"):
    import concourse.bacc as bacc
    import concourse.mybir as mybir
    import concourse.tile as tile
    from concourse.masks import make_identity
    f32 = mybir.dt.float32

    nc = bacc.Bacc("TRN2", num_swdge_queues=NQ)
    feat_pad = nc.declare_dram_parameter("feat_pad", [TBL_ROWS, D], f32, isOutput=False)
    f0T_in = nc.declare_dram_parameter("f0T", [D, NCOLS], f32, isOutput=False)
    idx_in = nc.declare_dram_parameter("idx_all", [128, SLOTS_TOT // 16], mybir.dt.int16, isOutput=False)
    dcol_in = nc.declare_dram_parameter("dcol", [128, NTILE_TOT], f32, isOutput=False)
    wval_in = nc.declare_dram_parameter("wval", [128, NTILE_TOT], f32, isOutput=False)
    iota_in = nc.declare_dram_parameter("iota", [128, WIN], f32, isOutput=False)
    W_in = nc.declare_dram_parameter("Wflat", [D, L * 3 * D], f32, isOutput=False)
    b_in = nc.declare_dram_parameter("bvec", [D, L], f32, isOutput=False)
    pw_in = nc.declare_dram_parameter("pwv", [D, 1], f32, isOutput=False)
    pb_in = nc.declare_dram_parameter("pbv", [1, 1], f32, isOutput=False)
    y_out = nc.declare_dram_parameter("y", [NCOLS, 1], f32, isOutput=True)

    ag = {}
    for nm in ("u1", "h1", "u1b"):
        ag[nm] = (
            nc.dram_tensor(f"agin_{nm}", [NCOLS, D], f32),
            nc.dram_tensor(f"agout_{nm}", [TBL_ROWS, D], f32, addr_space="Shared"),
        )

    NGW = GMAX // 128            # gather tile width (128-slot cols)
    GPH = (CWIN * SLOTS_W + GMAX - 1) // GMAX   # gathers per chunk-half

    with tile.TileContext(nc) as tc:
        with (
            tc.tile_pool(name="const", bufs=1) as cp,
            tc.tile_pool(name="gbuf", bufs=4) as gp,
            tc.tile_pool(name="mbuf", bufs=2) as mp,
            tc.tile_pool(name="sT", bufs=1) as sp,
            tc.tile_pool(name="rows", bufs=1) as rp,
            tc.tile_pool(name="small", bufs=2) as ip,
            tc.tile_pool(name="spsum", bufs=2, space="PSUM") as pp,
            tc.tile_pool(name="opsum", bufs=2, space="PSUM") as tp,
            tc.tile_pool(name="hpsum", bufs=1, space="PSUM") as hp_pool,
        ):
            dcol = cp.tile([128, NTILE_TOT], f32)
            nc.sync.dma_start(out=dcol[:], in_=dcol_in[:])
            wval = cp.tile([128, NTILE_TOT], f32)
            nc.sync.dma_start(out=wval[:], in_=wval_in[:])
            iota = cp.tile([128, WIN], f32)
            nc.sync.dma_start(out=iota[:], in_=iota_in[:])
            idxt = cp.tile([128, SLOTS_TOT // 16], mybir.dt.int16)
            nc.sync.dma_start(out=idxt[:], in_=idx_in[:])
            wfl = cp.tile([D, L * 3 * D], f32)
            nc.sync.dma_start(out=wfl[:], in_=W_in[:])
            bv = cp.tile([D, L], f32)
            nc.sync.dma_start(out=bv[:], in_=b_in[:])
            pwv = cp.tile([D, 1], f32)
            nc.sync.dma_start(out=pwv[:], in_=pw_in[:])
            pbv = cp.tile([1, 1], f32)
            nc.sync.dma_start(out=pbv[:], in_=pb_in[:])
            f0T = cp.tile([D, NCOLS], f32)
            nc.sync.dma_start(out=f0T[:], in_=f0T_in[:])
            ident = cp.tile([128, 128], f32)
            make_identity(nc, ident[:])

            u1T = sp.tile([D, NCOLS], f32, tag="u1T")
            if mode == "full":
                h1T = sp.tile([D, NCOLS], f32, tag="h1T")

            gq = [0]

            def chunk_gathers(table, c, tag):
                """Issue the gathers for chunk c; returns (glo, ghi) tiles
                shaped [128, NGW*GPH, D] (cols beyond the chunk's slot count
                are unused)."""
                nw = CHUNK_WINS[c]
                G = nw * SLOTS_W
                base = int(CH_SLOT_BASE[c])
                out = []
                for hx in (0, 1):
                    tab = table[hx * HALF:(hx + 1) * HALF, :]
                    gt = gp.tile([128, NGW * GPH, D], f32, tag=f"g{hx}",
                                 name=f"g{hx}_{tag}_{c}")
                    o = 0
                    while o < G:
                        g = min(GMAX, G - o)
                        off = (base + hx * G + o) // 16
                        nc.gpsimd.dma_gather(
                            gt[:, o // 128:(o + g) // 128, :],
                            tab,
                            idxt[:, off:off + g // 16],
                            g, g, D,
                            queue_num=gq[0] % NQ,
                        )
                        gq[0] += 1
                        o += GMAX
                    out.append(gt)
                return out

            def spmm_chunk(table, c, tag):
                """Gathers + mask gen + PE reduce for chunk c. Returns psum
                tile [64, nw*WIN] (caller evacuates / consumes)."""
                nw = CHUNK_WINS[c]
                glo, ghi = chunk_gathers(table, c, tag)
                nt = 2 * nw * TPW
                tb = int(CH_TILE_BASE[c])
                mask = mp.tile([128, 2 * CWIN * TPW * WIN], f32, tag="mask",
                               name=f"mask_{tag}_{c}")
                m3 = mask[:, :nt * WIN].rearrange("p (t o) -> p t o", o=WIN)
                i3 = iota[:].rearrange("p (o t) -> p o t", o=1).to_broadcast(
                    [128, nt, WIN])
                d3 = dcol[:, tb:tb + nt].rearrange(
                    "p (t o) -> p t o", o=1).to_broadcast([128, nt, WIN])
                w3 = wval[:, tb:tb + nt].rearrange(
                    "p (t o) -> p t o", o=1).to_broadcast([128, nt, WIN])
                nc.vector.tensor_tensor(out=m3, in0=i3, in1=d3,
                                        op=mybir.AluOpType.is_equal)
                nc.vector.tensor_tensor(out=m3, in0=m3, in1=w3,
                                        op=mybir.AluOpType.mult)
                ps = pp.tile([64, CWIN * WIN], f32, tag="spsum", name=f"ps_{tag}_{c}")
                for wi in range(nw):
                    for j in range(2 * TPW):
                        hx, t = j // TPW, j % TPW
                        gt = glo if hx == 0 else ghi
                        lhsT = gt[:, wi * TPW + t, :]
                        mt = hx * (nw * TPW) + wi * TPW + t
                        nc.tensor.matmul(
                            ps[:, WIN * wi:WIN * (wi + 1)],
                            lhsT,
                            mask[:, mt * WIN:(mt + 1) * WIN],
                            start=(j == 0), stop=(j == 2 * TPW - 1),
                        )
                return ps

            def transpose_blocks(sT, rows, c, tag):
                """PE-transpose the 128-col blocks covered by chunk c of sT
                into rows[:, k, :]."""
                k0 = int(CH_COL_BASE[c]) // 128
                k1 = int(CH_COL_BASE[c + 1]) // 128
                for k in range(k0, k1):
                    tps = tp.tile([128, D], f32, tag="tpsum", name=f"tps_{tag}_{k}")
                    nc.tensor.transpose(tps[:], sT[:, k * 128:(k + 1) * 128],
                                        ident[:64, :64])
                    nc.vector.tensor_copy(out=rows[:, k, :], in_=tps[:])

            def spmm(table, out_sT, tag, rows=None):
                for c in range(NCH):
                    ps = spmm_chunk(table, c, tag)
                    col0 = int(CH_COL_BASE[c])
                    ncol = CHUNK_WINS[c] * WIN
                    nc.vector.tensor_copy(
                        out=out_sT[:, col0:col0 + ncol], in_=ps[:, :ncol])
                    if rows is not None:
                        transpose_blocks(out_sT, rows, c, tag)

            def spmm_fused_dense(table, l, u0T, u1T_, outT, tag, rows=None):
                """SpMM for U2 fused with the dense layer; layer 2 adds the
                prediction head."""
                for c in range(NCH):
                    ps = spmm_chunk(table, c, tag)
                    col0 = int(CH_COL_BASE[c])
                    ncol = CHUNK_WINS[c] * WIN
                    u2c = mp.tile([64, CWIN * WIN], f32, tag="u2c", name=f"u2c_{tag}_{c}")
                    nc.vector.tensor_copy(out=u2c[:, :ncol], in_=ps[:, :ncol])
                    dp = tp.tile([64, CWIN * WIN], f32, tag="dpsum", name=f"dp_{tag}_{c}")
                    for t, uT in enumerate((u0T, u1T_, u2c)):
                        rhs = uT[:, :ncol] if t == 2 else uT[:, col0:col0 + ncol]
                        nc.tensor.matmul(
                            dp[:, :ncol],
                            wfl[:, (l * 3 + t) * D:(l * 3 + t + 1) * D],
                            rhs,
                            start=(t == 0), stop=(t == 2),
                        )
                    if outT is None:
                        # layer-2 output is only read by the head below —
                        # keep it chunk-local instead of a [D, NCOLS] table
                        oc = mp.tile([64, CWIN * WIN], f32, tag="h2c",
                                     name=f"h2c_{tag}_{c}")
                        osl = oc[:, :ncol]
                    else:
                        osl = outT[:, col0:col0 + ncol]
                    nc.scalar.activation(
                        out=osl,
                        in_=dp[:, :ncol],
                        func=mybir.ActivationFunctionType.Relu,
                        bias=bv[:, l:l + 1],
                        scale=1.0,
                    )
                    if rows is not None:
                        transpose_blocks(outT, rows, c, tag)
                    if l == L - 1:
                        hp = hp_pool.tile([1, CWIN * WIN], f32, tag="hpsum",
                                          name=f"hp_{c}")
                        nc.tensor.matmul(
                            hp[:, :ncol], pwv[:],
                            osl,
                            start=True, stop=True)
                        yc = ip.tile([1, CWIN * WIN], f32, tag="yc", name=f"yc_{c}")
                        nc.vector.tensor_scalar(
                            out=yc[:1, :ncol], in0=hp[:, :ncol],
                            scalar1=pbv[:1, :1],
                            scalar2=None, op0=mybir.AluOpType.add)
                        nc.sync.dma_start(
                            out=y_out[col0:col0 + ncol, :],
                            in_=yc[:1, :ncol])

            def rows_tile(tag):
                return rp.tile([128, NBLK, D], f32, tag="rows", name=f"rows_{tag}")

            def flush_rows(rows, agin):
                nc.sync.dma_start(
                    out=agin.ap().rearrange("(p k) d -> p k d", p=128),
                    in_=rows[:])

            def allgather(nm):
                agin, agout = ag[nm]
                nc.gpsimd.collective_compute(
                    "AllGather",
                    mybir.AluOpType.bypass,
                    ins=[agin.ap().opt()],
                    outs=[agout.ap().opt()],
                    replica_groups=[list(range(NCORES))],
                )

            if mode == "spmm_only":
                for r in range(repeat):
                    spmm(feat_pad, u1T, f"r{r}s1")
                nc.sync.dma_start(out=y_out[:NSH, :], in_=u1T[:1, :NSH])
            elif mode == "gather_only":
                nc.vector.memset(u1T[:1, :], 0.0)
                for r in range(repeat):
                    for c in range(NCH):
                        glo, ghi = chunk_gathers(feat_pad, c, f"r{r}")
                        nc.vector.tensor_copy(out=u1T[:1, c * 16:(c + 1) * 16],
                                              in_=glo[:1, 0, :16])
                nc.sync.dma_start(out=y_out[:NSH, :], in_=u1T[:1, :NSH])
            elif mode == "ag_only":
                for r in range(repeat):
                    rows = rows_tile(f"r{r}t1")
                    for c in range(NCH):
                        transpose_blocks(f0T, rows, c, f"r{r}t1")
                    flush_rows(rows, ag["u1"][0])
                    allgather("u1")
                nc.sync.dma_start(out=y_out[:NSH, :], in_=f0T[:1, :NSH])
            elif mode == "tr_only":
                for r in range(repeat):
                    rows = rows_tile(f"r{r}t1")
                    for c in range(NCH):
                        transpose_blocks(f0T, rows, c, f"r{r}t1")
                    flush_rows(rows, ag["u1"][0])
                nc.sync.dma_start(out=y_out[:NSH, :], in_=f0T[:1, :NSH])
            if mode != "full":
                repeat = 0
            for r in range(repeat):
                # ---- layer 1 ----
                rows = rows_tile(f"r{r}t1")
                spmm(feat_pad, u1T, f"r{r}s1", rows=rows)
                flush_rows(rows, ag["u1"][0])
                allgather("u1")
                # h1T <- relu(f0 Wa + u1 Wb + u2 Wc + b0), u2 fused from SpMM2
                rows = rows_tile(f"r{r}t2")
                spmm_fused_dense(ag["u1"][1], 0, f0T, u1T, h1T, f"r{r}s2",
                                 rows=rows)
                flush_rows(rows, ag["h1"][0])
                allgather("h1")
                # ---- layer 2 ----
                rows = rows_tile(f"r{r}t3")
                spmm(ag["h1"][1], u1T, f"r{r}s3", rows=rows)
                flush_rows(rows, ag["u1b"][0])
                allgather("u1b")
                spmm_fused_dense(ag["u1b"][1], 1, h1T, u1T, None, f"r{r}s4")

    _finalize_with_split(nc)
    return nc


def _get_runner():
    if "runner" in _CACHE:
        return _CACHE["runner"]
    nc = _build_nc()
    _CACHE["runner"] = _build_runner(nc, NCORES)
    return _CACHE["runner"]


def kernel(features, src, dst, W, b, pw, pb):
    in_maps = _host_prep(features, src, dst, W, b, pw, pb)
    run = _get_runner()
    results, times = run(in_maps, iters=1)
    _CACHE["last_times"] = times
    col_of = _CACHE["col_of"]
    y = np.empty((N, 1), dtype=np.float32)
    for i in range(NCORES):
        yc = results[i]["y"].reshape(NCOLS)
        own = slice(i * NSH, (i + 1) * NSH)
        y[own, 0] = yc[col_of[own]]
    return y


# revision 16
# speedup vs baseline: 1.0167x; 1.0167x over previous
"""ChebNet (K=3, L=2) forward on 8 Trainium2 NeuronCores — v2.

Node-sharded SPMD: each core owns 6250 dst rows, placed into 6784 padded
columns (53 blocks x 128) by a 2-D greedy packing that balances each
32-col window's in-edge count per source half (halves = core groups 0-3 /
4-7, each half's table slice int16-addressable). Chebyshev recurrence in
the U-basis (U1 = A_hat U0, U2 = A_hat U1; out = U0(W0-W2) - U1 W1 +
2 U2 W2).

SpMM: per PSUM chunk (16 windows = 512 cols), one dma_gather per half
(up to 4096 edge slots; 2 tiles of 128 slots per window-half, static) +
PE mask-matmuls with DVE-generated masks carrying norm[src]*norm[dst].
Table layout is p-major (table row = p*53 + k for column k*128+p) so the
post-SpMM transpose writes DRAM with 128 fat descriptors. idx/dcol/wval
stay SBUF-resident. Transposes are fused into the producing SpMM's chunk
loop; 3 AllGathers rebuild the node table between SpMMs.
"""
import os
import numpy as np

N, E, D, K, L = 50000, 800000, 64, 3, 2
NCORES = 8
NSH = N // NCORES              # 6250 owned nodes per core
NBLK = 53                      # 128-col blocks per core
NCOLS = NBLK * 128             # 6784 padded columns
WIN = 32                       # dst cols per window
NWIN = NCOLS // WIN            # 212
TPW = 2                        # tiles per (window, half) — static (cap 256)
TBL_ROWS = NCORES * NCOLS      # 54272
HALF = TBL_ROWS // 2           # 27136 rows per half (int16-reachable)
CWIN = 16                      # windows per PSUM chunk
CHUNK_WINS = [CWIN] * 13 + [NWIN - 13 * CWIN]   # 13x16 + 1x4
NCH = len(CHUNK_WINS)          # 14
SLOTS_W = TPW * 128            # 256 slots per window-half
CH_SLOT_BASE = np.concatenate([[0], np.cumsum([2 * nw * SLOTS_W for nw in CHUNK_WINS])]).astype(int)
CH_TILE_BASE = np.concatenate([[0], np.cumsum([2 * nw * TPW for nw in CHUNK_WINS])]).astype(int)
CH_COL_BASE = np.concatenate([[0], np.cumsum([nw * WIN for nw in CHUNK_WINS])]).astype(int)
SLOTS_TOT = int(CH_SLOT_BASE[-1])   # 108544
NTILE_TOT = SLOTS_TOT // 128        # 848
GMAX = int(os.environ.get("KGMAX", "1024"))  # max rows per dma_gather call
                                             # (>1024 overflows the SWDGE ring)
NQ = int(os.environ.get("KQ", "4"))          # SWDGE queues to rotate over

_CACHE = {}


# ---------------------------------------------------------------------------
# Workaround for this walrus build: any instruction carrying >1 sync wait is
# rejected ("Too many sync wait commands"). Hoist extras onto 1-wait NoOps on
# the same engine (per-engine program order preserves semantics).
_ws_counter = [0]


def _split_multiwaits(nc):
    import concourse.mybir as mybir
    n_split = 0
    for fn in nc.m.functions:
        for bb in fn.blocks:
            new_list = []
            changed = False
            for inst in bb.instructions:
                si = inst.sync_info
                waits = list(si.on_wait) if si is not None else []
                if len(waits) > 1:
                    changed = True
                    for w in waits[:-1]:
                        _ws_counter[0] += 1
                        nop = mybir.InstNoOp(
                            name=f"waitsplit-{_ws_counter[0]}",
                            ins=[], outs=[],
                            sync_info=mybir.SyncInfo(on_wait=[w], on_update=[]),
                        )
                        nop.engine = inst.engine
                        nc.register_instruction(nop, overwrite=True)
                        new_list.append(nop)
                        n_split += 1
                    si.on_wait = waits[-1:]
                new_list.append(inst)
            if changed:
                bb.instructions[:] = new_list
    return n_split


def _finalize_with_split(nc):
    import concourse.bass as _bass
    nc.compile()           # Bacc passes (incl. library-load insertion)
    _split_multiwaits(nc)  # after replace_nops_with_events, before freeze
    _bass.Bass.finalize(nc)


def _build_runner(nc, n_cores):
    """SPMD runner over the axon PJRT backend (keeps the jitted executable
    and device-resident inputs so repeat calls can be timed)."""
    import jax
    from jax.sharding import Mesh, PartitionSpec
    from jax.experimental.shard_map import shard_map
    import concourse.mybir as mybir
    from concourse.bass2jax import (
        _bass_exec_p, install_neuronx_cc_hook, partition_id_tensor)

    install_neuronx_cc_hook()
    partition_name = nc.partition_id_tensor.name if nc.partition_id_tensor else None

    in_names, out_names, out_avals, zero_outs = [], [], [], []
    for alloc in nc.m.functions[0].allocations:
        if not isinstance(alloc, mybir.MemoryLocationSet):
            continue
        name = alloc.memorylocations[0].name
        if alloc.kind == "ExternalInput":
            if name != partition_name:
                in_names.append(name)
        elif alloc.kind == "ExternalOutput":
            shape = tuple(alloc.tensor_shape)
            dtype = mybir.dt.np(alloc.dtype)
            out_names.append(name)
            out_avals.append(jax.core.ShapedArray(shape, dtype))
            zero_outs.append(np.zeros(shape, dtype))
    n_params = len(in_names)
    all_in_names = list(in_names) + list(out_names)
    if partition_name is not None:
        all_in_names.append(partition_name)

    def _body(*args):
        operands = list(args)
        if partition_name is not None:
            operands.append(partition_id_tensor())
        outs = _bass_exec_p.bind(
            *operands,
            out_avals=tuple(out_avals),
            in_names=tuple(all_in_names),
            out_names=tuple(out_names),
            lowering_input_output_aliases=(),
            sim_require_finite=True,
            sim_require_nnan=True,
            nc=nc,
        )
        return tuple(outs)

    devices = jax.devices()[:n_cores]
    mesh = Mesh(np.asarray(devices), ("core",))
    in_specs = (PartitionSpec("core"),) * (n_params + len(out_names))
    out_specs = (PartitionSpec("core"),) * len(out_names)
    sharded = jax.jit(
        shard_map(_body, mesh=mesh, in_specs=in_specs, out_specs=out_specs,
                  check_rep=False),
        keep_unused=True,
    )

    def run(in_maps, iters=1):
        import time as _time
        per_core = [[np.asarray(m[name]) for name in in_names] for m in in_maps]
        concat_in = [
            np.concatenate([per_core[c][i] for c in range(n_cores)], axis=0)
            for i in range(n_params)
        ]
        concat_zeros = [
            np.zeros((n_cores * z.shape[0], *z.shape[1:]), z.dtype)
            for z in zero_outs
        ]
        sharding = jax.sharding.NamedSharding(mesh, PartitionSpec("core"))
        dev_in = [jax.device_put(a, sharding) for a in concat_in + concat_zeros]
        out = sharded(*dev_in)
        jax.block_until_ready(out)
        times = []
        for _ in range(iters):
            t0 = _time.perf_counter()
            out = sharded(*dev_in)
            jax.block_until_ready(out)
            times.append(_time.perf_counter() - t0)
        results = [
            {name: np.asarray(out[i]).reshape(n_cores, *out_avals[i].shape)[c]
             for i, name in enumerate(out_names)}
            for c in range(n_cores)
        ]
        return results, times

    return run


def _pack_windows(degA, degB):
    """Greedy 2-D balanced packing of NSH nodes into NWIN windows of <=32,
    minimizing the max per-half load. Returns win_of[node]."""
    order = np.argsort(-(degA + degB), kind="stable")
    loadA = np.zeros(NWIN)
    loadB = np.zeros(NWIN)
    cnt = np.zeros(NWIN, np.int64)
    win_of = np.zeros(degA.size, np.int64)
    for v in order:
        av, bv = degA[v], degB[v]
        cand = np.maximum(loadA + av, loadB + bv) + 1e-3 * (loadA + loadB)
        cand[cnt >= 32] = np.inf
        w = int(np.argmin(cand))
        win_of[v] = w
        loadA[w] += av
        loadB[w] += bv
        cnt[w] += 1
    if loadA.max() > SLOTS_W or loadB.max() > SLOTS_W:
        raise RuntimeError(f"window overflow: {loadA.max()}, {loadB.max()}")
    return win_of


def _host_prep(features, src, dst, W, b, pw, pb):
    src = np.asarray(src).astype(np.int64)
    dst = np.asarray(dst).astype(np.int64)
    features = np.asarray(features, dtype=np.float32)
    W = np.asarray(W, dtype=np.float32)
    b = np.asarray(b, dtype=np.float32)
    pw = np.asarray(pw, dtype=np.float32).reshape(D, 1)
    pb = np.asarray(pb, dtype=np.float32).reshape(1)

    deg = np.bincount(dst, minlength=N).astype(np.float32)
    norm = np.clip(deg, 1.0, None) ** -0.5
    wedge = (norm[src] * norm[dst]).astype(np.float32)

    half_of_src = (src >= N // 2).astype(np.int64)   # half = src core group
    degA = np.bincount(dst[half_of_src == 0], minlength=N)
    degB = np.bincount(dst[half_of_src == 1], minlength=N)

    # --- per-core window packing -> global column/table-row maps ----------
    col_of = np.zeros(N, np.int64)       # local column on owner core
    for i in range(NCORES):
        sl = slice(i * NSH, (i + 1) * NSH)
        win_of = _pack_windows(degA[sl].astype(np.float64),
                               degB[sl].astype(np.float64))
        # positions within each window in node order
        order = np.argsort(win_of, kind="stable")
        pos = np.arange(NSH) - np.searchsorted(win_of[order], win_of[order])
        lc = np.empty(NSH, np.int64)
        lc[order] = win_of[order] * WIN + pos
        col_of[sl] = lc
    core_of = np.arange(N) // NSH
    p_of = col_of % 128
    k_of = col_of // 128
    table_row = core_of * NCOLS + p_of * NBLK + k_of

    feat_pad = np.zeros((TBL_ROWS, D), dtype=np.float32)
    feat_pad[table_row] = features

    Wflat = np.zeros((D, L * 3 * D), dtype=np.float32)
    for l in range(L):
        for t, Wt in enumerate((W[l, 0] - W[l, 2], -W[l, 1], 2.0 * W[l, 2])):
            Wflat[:, (l * 3 + t) * D:(l * 3 + t + 1) * D] = Wt

    # window -> chunk map
    chunk_of_w = np.zeros(NWIN, np.int64)
    wl_of_w = np.zeros(NWIN, np.int64)
    w0 = 0
    for c, nw in enumerate(CHUNK_WINS):
        chunk_of_w[w0:w0 + nw] = c
        wl_of_w[w0:w0 + nw] = np.arange(nw)
        w0 += nw

    gather_sizes = []
    for c, nw in enumerate(CHUNK_WINS):
        G = nw * SLOTS_W
        for h in (0, 1):
            o = 0
            while o < G:
                gather_sizes.append(min(GMAX, G - o))
                o += GMAX

    in_maps = []
    for i in range(NCORES):
        sel = core_of[dst] == i
        e_src = src[sel]
        e_w = wedge[sel]
        lc_d = col_of[dst[sel]]
        win = lc_d // WIN
        dcol = (lc_d % WIN).astype(np.float32)
        h = half_of_src[sel]

        key = win * 2 + h
        order = np.argsort(key, kind="stable")
        ks = key[order]
        grp_start = np.searchsorted(ks, np.arange(NWIN * 2), side="left")
        grp_cnt = np.diff(np.append(grp_start, ks.size))
        if grp_cnt.max() > SLOTS_W:
            raise RuntimeError(f"slot overflow core {i}: {grp_cnt.max()}")
        rank = np.arange(ks.size) - grp_start[ks]

        ww = win[order]
        hh = h[order]
        c_ = chunk_of_w[ww]
        wl = wl_of_w[ww]
        nw_c = np.array(CHUNK_WINS)[c_]
        slot = (CH_SLOT_BASE[c_] + hh * nw_c * SLOTS_W + wl * SLOTS_W + rank)
        tile = (CH_TILE_BASE[c_] + hh * nw_c * TPW + wl * TPW + rank // 128)
        part = rank % 128

        idx_slots = np.zeros(SLOTS_TOT, dtype=np.int16)
        idx_slots[slot] = (table_row[e_src[order]] - hh * HALF).astype(np.int16)
        dcol_slots = np.full(SLOTS_TOT, -1.0, dtype=np.float32)
        wval_slots = np.zeros(SLOTS_TOT, dtype=np.float32)
        dcol_slots[tile * 128 + part] = dcol[order]
        wval_slots[tile * 128 + part] = e_w[order]

        # idx wrap: per gather, slot j -> idx_arr[j%16, base + j//16]
        idx_arr = np.zeros((16, SLOTS_TOT // 16), dtype=np.int16)
        pos = 0
        for G in gather_sizes:
            blk = idx_slots[pos:pos + G].reshape(G // 16, 16).T
            idx_arr[:, pos // 16:(pos + G) // 16] = blk
            pos += G
        idx_arr = np.tile(idx_arr, (8, 1))

        dcl = dcol_slots.reshape(NTILE_TOT, 128).T.copy()
        wvl = wval_slots.reshape(NTILE_TOT, 128).T.copy()

        f0T = np.zeros((D, NCOLS), dtype=np.float32)
        own = slice(i * NSH, (i + 1) * NSH)
        f0T[:, col_of[own]] = features[own].T

        iota = np.tile(np.arange(WIN, dtype=np.float32)[None, :], (128, 1))

        in_maps.append({
            "feat_pad": feat_pad,
            "f0T": f0T,
            "idx_all": idx_arr,
            "dcol": dcl,
            "wval": wvl,
            "iota": iota,
            "Wflat": Wflat,
            "bvec": b.T.copy(),
            "pwv": pw,
            "pbv": pb.reshape(1, 1),
        })
    _CACHE["col_of"] = col_of
    return in_maps


def _build_nc(repeat=1, mode="full"):
    import concourse.bacc as bacc
    import concourse.mybir as mybir
    import concourse.tile as tile
    from concourse.masks import make_identity
    f32 = mybir.dt.float32

    nc = bacc.Bacc("TRN2", num_swdge_queues=NQ)
    feat_pad = nc.declare_dram_parameter("feat_pad", [TBL_ROWS, D], f32, isOutput=False)
    f0T_in = nc.declare_dram_parameter("f0T", [D, NCOLS], f32, isOutput=False)
    idx_in = nc.declare_dram_parameter("idx_all", [128, SLOTS_TOT // 16], mybir.dt.int16, isOutput=False)
    dcol_in = nc.declare_dram_parameter("dcol", [128, NTILE_TOT], f32, isOutput=False)
    wval_in = nc.declare_dram_parameter("wval", [128, NTILE_TOT], f32, isOutput=False)
    iota_in = nc.declare_dram_parameter("iota", [128, WIN], f32, isOutput=False)
    W_in = nc.declare_dram_parameter("Wflat", [D, L * 3 * D], f32, isOutput=False)
    b_in = nc.declare_dram_parameter("bvec", [D, L], f32, isOutput=False)
    pw_in = nc.declare_dram_parameter("pwv", [D, 1], f32, isOutput=False)
    pb_in = nc.declare_dram_parameter("pbv", [1, 1], f32, isOutput=False)
    y_out = nc.declare_dram_parameter("y", [NCOLS, 1], f32, isOutput=True)

    ag = {}
    for nm in ("u1", "h1", "u1b"):
        ag[nm] = (
            nc.dram_tensor(f"agin_{nm}", [NCOLS, D], f32),
            nc.dram_tensor(f"agout_{nm}", [TBL_ROWS, D], f32, addr_space="Shared"),
        )

    NGW = GMAX // 128            # gather tile width (128-slot cols)
    GPH = (CWIN * SLOTS_W + GMAX - 1) // GMAX   # gathers per chunk-half

    with tile.TileContext(nc) as tc:
        with (
            tc.tile_pool(name="const", bufs=1) as cp,
            tc.tile_pool(name="gbuf", bufs=3) as gp,
            tc.tile_pool(name="mbuf", bufs=2) as mp,
            tc.tile_pool(name="sT", bufs=1) as sp,
            tc.tile_pool(name="rows", bufs=1) as rp,
            tc.tile_pool(name="small", bufs=2) as ip,
            tc.tile_pool(name="spsum", bufs=2, space="PSUM") as pp,
            tc.tile_pool(name="opsum", bufs=2, space="PSUM") as tp,
            tc.tile_pool(name="hpsum", bufs=1, space="PSUM") as hp_pool,
        ):
            dcol = cp.tile([128, NTILE_TOT], f32)
            nc.sync.dma_start(out=dcol[:], in_=dcol_in[:])
            wval = cp.tile([128, NTILE_TOT], f32)
            nc.sync.dma_start(out=wval[:], in_=wval_in[:])
            iota = cp.tile([128, WIN], f32)
            nc.sync.dma_start(out=iota[:], in_=iota_in[:])
            idxt = cp.tile([128, SLOTS_TOT // 16], mybir.dt.int16)
            nc.sync.dma_start(out=idxt[:], in_=idx_in[:])
            wfl = cp.tile([D, L * 3 * D], f32)
            nc.sync.dma_start(out=wfl[:], in_=W_in[:])
            bv = cp.tile([D, L], f32)
            nc.sync.dma_start(out=bv[:], in_=b_in[:])
            pwv = cp.tile([D, 1], f32)
            nc.sync.dma_start(out=pwv[:], in_=pw_in[:])
            pbv = cp.tile([1, 1], f32)
            nc.sync.dma_start(out=pbv[:], in_=pb_in[:])
            f0T = cp.tile([D, NCOLS], f32)
            nc.sync.dma_start(out=f0T[:], in_=f0T_in[:])
            ident = cp.tile([128, 128], f32)
            make_identity(nc, ident[:])

            u1T = sp.tile([D, NCOLS], f32, tag="u1T")
            if mode == "full":
                h1T = sp.tile([D, NCOLS], f32, tag="h1T")

            gq = [0]

            def chunk_gathers(table, c, tag):
                """Issue the gathers for chunk c; returns (glo, ghi) tiles
                shaped [128, NGW*GPH, D] (cols beyond the chunk's slot count
                are unused)."""
                nw = CHUNK_WINS[c]
                G = nw * SLOTS_W
                base = int(CH_SLOT_BASE[c])
                out = []
                for hx in (0, 1):
                    tab = table[hx * HALF:(hx + 1) * HALF, :]
                    gt = gp.tile([128, NGW * GPH, D], f32, tag=f"g{hx}",
                                 name=f"g{hx}_{tag}_{c}")
                    o = 0
                    while o < G:
                        g = min(GMAX, G - o)
                        off = (base + hx * G + o) // 16
                        nc.gpsimd.dma_gather(
                            gt[:, o // 128:(o + g) // 128, :],
                            tab,
                            idxt[:, off:off + g // 16],
                            g, g, D,
                            queue_num=gq[0] % NQ,
                        )
                        gq[0] += 1
                        o += GMAX
                    out.append(gt)
                return out

            def spmm_chunk(table, c, tag):
                """Gathers + mask gen + PE reduce for chunk c. Returns psum
                tile [64, nw*WIN] (caller evacuates / consumes)."""
                nw = CHUNK_WINS[c]
                glo, ghi = chunk_gathers(table, c, tag)
                nt = 2 * nw * TPW
                tb = int(CH_TILE_BASE[c])
                mask = mp.tile([128, 2 * CWIN * TPW * WIN], f32, tag="mask",
                               name=f"mask_{tag}_{c}")
                m3 = mask[:, :nt * WIN].rearrange("p (t o) -> p t o", o=WIN)
                i3 = iota[:].rearrange("p (o t) -> p o t", o=1).to_broadcast(
                    [128, nt, WIN])
                d3 = dcol[:, tb:tb + nt].rearrange(
                    "p (t o) -> p t o", o=1).to_broadcast([128, nt, WIN])
                w3 = wval[:, tb:tb + nt].rearrange(
                    "p (t o) -> p t o", o=1).to_broadcast([128, nt, WIN])
                nc.vector.tensor_tensor(out=m3, in0=i3, in1=d3,
                                        op=mybir.AluOpType.is_equal)
                nc.vector.tensor_tensor(out=m3, in0=m3, in1=w3,
                                        op=mybir.AluOpType.mult)
                ps = pp.tile([64, CWIN * WIN], f32, tag="spsum", name=f"ps_{tag}_{c}")
                for wi in range(nw):
                    for j in range(2 * TPW):
                        hx, t = j // TPW, j % TPW
                        gt = glo if hx == 0 else ghi
                        lhsT = gt[:, wi * TPW + t, :]
                        mt = hx * (nw * TPW) + wi * TPW + t
                        nc.tensor.matmul(
                            ps[:, WIN * wi:WIN * (wi + 1)],
                            lhsT,
                            mask[:, mt * WIN:(mt + 1) * WIN],
                            start=(j == 0), stop=(j == 2 * TPW - 1),
                        )
                return ps

            def transpose_blocks(sT, rows, c, tag):
                """PE-transpose the 128-col blocks covered by chunk c of sT
                into rows[:, k, :]."""
                k0 = int(CH_COL_BASE[c]) // 128
                k1 = int(CH_COL_BASE[c + 1]) // 128
                for k in range(k0, k1):
                    tps = tp.tile([128, D], f32, tag="tpsum", name=f"tps_{tag}_{k}")
                    nc.tensor.transpose(tps[:], sT[:, k * 128:(k + 1) * 128],
                                        ident[:64, :64])
                    nc.vector.tensor_copy(out=rows[:, k, :], in_=tps[:])

            def spmm(table, out_sT, tag, rows=None):
                for c in range(NCH):
                    ps = spmm_chunk(table, c, tag)
                    col0 = int(CH_COL_BASE[c])
                    ncol = CHUNK_WINS[c] * WIN
                    nc.vector.tensor_copy(
                        out=out_sT[:, col0:col0 + ncol], in_=ps[:, :ncol])
                    if rows is not None:
                        transpose_blocks(out_sT, rows, c, tag)

            def spmm_fused_dense(table, l, u0T, u1T_, outT, tag, rows=None):
                """SpMM for U2 fused with the dense layer; layer 2 adds the
                prediction head."""
                for c in range(NCH):
                    ps = spmm_chunk(table, c, tag)
                    col0 = int(CH_COL_BASE[c])
                    ncol = CHUNK_WINS[c] * WIN
                    u2c = mp.tile([64, CWIN * WIN], f32, tag="u2c", name=f"u2c_{tag}_{c}")
                    nc.vector.tensor_copy(out=u2c[:, :ncol], in_=ps[:, :ncol])
                    dp = tp.tile([64, CWIN * WIN], f32, tag="dpsum", name=f"dp_{tag}_{c}")
                    for t, uT in enumerate((u0T, u1T_, u2c)):
                        rhs = uT[:, :ncol] if t == 2 else uT[:, col0:col0 + ncol]
                        nc.tensor.matmul(
                            dp[:, :ncol],
                            wfl[:, (l * 3 + t) * D:(l * 3 + t + 1) * D],
                            rhs,
                            start=(t == 0), stop=(t == 2),
                        )
                    if outT is None:
                        # layer-2 output is only read by the head below —
                        # keep it chunk-local instead of a [D, NCOLS] table
                        oc = mp.tile([64, CWIN * WIN], f32, tag="h2c",
                                     name=f"h2c_{tag}_{c}")
                        osl = oc[:, :ncol]
                    else:
                        osl = outT[:, col0:col0 + ncol]
                    nc.scalar.activation(
                        out=osl,
                        in_=dp[:, :ncol],
                        func=mybir.ActivationFunctionType.Relu,
                        bias=bv[:, l:l + 1],
                        scale=1.0,
                    )
                    if rows is not None:
                        transpose_blocks(outT, rows, c, tag)
                    if l == L - 1:
                        hp = hp_pool.tile([1, CWIN * WIN], f32, tag="hpsum",
                                          name=f"hp_{c}")
                        nc.tensor.matmul(
                            hp[:, :ncol], pwv[:],
                            osl,
                            start=True, stop=True)
                        yc = ip.tile([1, CWIN * WIN], f32, tag="yc", name=f"yc_{c}")
                        nc.vector.tensor_scalar(
                            out=yc[:1, :ncol], in0=hp[:, :ncol],
                            scalar1=pbv[:1, :1],
                            scalar2=None, op0=mybir.AluOpType.add)
                        nc.sync.dma_start(
                            out=y_out[col0:col0 + ncol, :],
                            in_=yc[:1, :ncol])

            def rows_tile(tag):
                return rp.tile([128, NBLK, D], f32, tag="rows", name=f"rows_{tag}")

            def flush_rows(rows, agin):
                nc.sync.dma_start(
                    out=agin.ap().rearrange("(p k) d -> p k d", p=128),
                    in_=rows[:])

            def allgather(nm):
                agin, agout = ag[nm]
                nc.gpsimd.collective_compute(
                    "AllGather",
                    mybir.AluOpType.bypass,
                    ins=[agin.ap().opt()],
                    outs=[agout.ap().opt()],
                    replica_groups=[list(range(NCORES))],
                )

            if mode == "spmm_only":
                for r in range(repeat):
                    spmm(feat_pad, u1T, f"r{r}s1")
                nc.sync.dma_start(out=y_out[:NSH, :], in_=u1T[:1, :NSH])
            elif mode == "gather_only":
                nc.vector.memset(u1T[:1, :], 0.0)
                for r in range(repeat):
                    for c in range(NCH):
                        glo, ghi = chunk_gathers(feat_pad, c, f"r{r}")
                        nc.vector.tensor_copy(out=u1T[:1, c * 16:(c + 1) * 16],
                                              in_=glo[:1, 0, :16])
                nc.sync.dma_start(out=y_out[:NSH, :], in_=u1T[:1, :NSH])
            elif mode == "ag_only":
                for r in range(repeat):
                    rows = rows_tile(f"r{r}t1")
                    for c in range(NCH):
                        transpose_blocks(f0T, rows, c, f"r{r}t1")
                    flush_rows(rows, ag["u1"][0])
                    allgather("u1")
                nc.sync.dma_start(out=y_out[:NSH, :], in_=f0T[:1, :NSH])
            elif mode == "tr_only":
                for r in range(repeat):
                    rows = rows_tile(f"r{r}t1")
                    for c in range(NCH):
                        transpose_blocks(f0T, rows, c, f"r{r}t1")
                    flush_rows(rows, ag["u1"][0])
                nc.sync.dma_start(out=y_out[:NSH, :], in_=f0T[:1, :NSH])
            if mode != "full":
                repeat = 0
            for r in range(repeat):
                # ---- layer 1 ----
                rows = rows_tile(f"r{r}t1")
                spmm(feat_pad, u1T, f"r{r}s1", rows=rows)
                flush_rows(rows, ag["u1"][0])
                allgather("u1")
                # h1T <- relu(f0 Wa + u1 Wb + u2 Wc + b0), u2 fused from SpMM2
                rows = rows_tile(f"r{r}t2")
                spmm_fused_dense(ag["u1"][1], 0, f0T, u1T, h1T, f"r{r}s2",
                                 rows=rows)
                flush_rows(rows, ag["h1"][0])
                allgather("h1")
                # ---- layer 2 ----
                rows = rows_tile(f"r{r}t3")
                spmm(ag["h1"][1], u1T, f"r{r}s3", rows=rows)
                flush_rows(rows, ag["u1b"][0])
                allgather("u1b")
                spmm_fused_dense(ag["u1b"][1], 1, h1T, u1T, None, f"r{r}s4")

    _finalize_with_split(nc)
    return nc


def _get_runner():
    if "runner" in _CACHE:
        return _CACHE["runner"]
    nc = _build_nc()
    _CACHE["runner"] = _build_runner(nc, NCORES)
    return _CACHE["runner"]


def kernel(features, src, dst, W, b, pw, pb):
    in_maps = _host_prep(features, src, dst, W, b, pw, pb)
    run = _get_runner()
    results, times = run(in_maps, iters=1)
    _CACHE["last_times"] = times
    col_of = _CACHE["col_of"]
    y = np.empty((N, 1), dtype=np.float32)
    for i in range(NCORES):
        yc = results[i]["y"].reshape(NCOLS)
        own = slice(i * NSH, (i + 1) * NSH)
        y[own, 0] = yc[col_of[own]]
    return y
